# revision 2
# baseline (speedup 1.0000x reference)
"""Trainium2 Bass kernel for nn_NestedConv (gnn_message_passing).

Math (per b, i):
    Xm      = X[b,i] * mask[b,i,:,None]                  # (N,D), rows k masked
    h1      = relu(Xm @ W1 + b1)                         # (N,D)
    h       = relu(h1 @ W2 + b2)                         # (N,D)
    out[b,i] = (A[b].T @ h) * mask[b,i,:,None]           # (N,D), rows j masked

Sharding: data-parallel over batch dim B=64 across 8 NeuronCores (8 b's each).

On-chip dataflow per (b, group of G=8 root nodes i):
  - X tiles loaded natural (k on partitions, (i,d) on free)
  - mask applied via one tensor_tensor mul against maskT columns
    (maskT[b,k,i] = mask[b,i,k], precomputed host-side) broadcast along d
  - PE transpose of tile-pairs [XmA|XmB] -> (d-stacked partitions, k free)
  - MLP layer 1: two quadrant-packed matmuls, stationary W1 (fp32r)
  - relu(+b1) on ScalarE, cast to bf16
  - MLP layer 2: stationary = h1^T slices (bf16), moving = W2 -> h natural (k,d)
  - message passing: stationary = A[b] (fp32r), moving = 8 h tiles batched
  - output mask fused into the PSUM->SBUF copy (tensor_tensor mul)
"""

import sys

sys.path.insert(0, "/opt/trn_rl_repo")

import numpy as np

B, N, D = 64, 128, 64
NC = 8
BSH = B // NC  # batches per core
G = 8  # root-node tiles per group
NG = N // G  # groups per batch

_built = {}


def _build(b2_nonzero: bool, cfg: dict, bsh: int = BSH, ng: int = NG):
    import concourse.bacc as bacc
    import concourse.mybir as mybir
    from concourse import tile
    from concourse.bass_interp import get_hw_module

    f32 = mybir.dt.float32
    f32r = mybir.dt.float32r
    bf16 = mybir.dt.bfloat16
    Relu = mybir.ActivationFunctionType.Relu

    mm1_r = cfg.get("mm1", "f32r") == "f32r"
    mm3_r = cfg.get("mm3", "f32r") == "f32r"
    mm2_bf = cfg.get("mm2", "bf16") == "bf16"
    w2dt = bf16 if mm2_bf else f32

    nc = bacc.Bacc("TRN2", target_bir_lowering=False, debug=False, num_devices=1)

    a_dt = f32r if mm3_r else f32
    w1_dt = f32r if mm1_r else f32
    X_d = nc.dram_tensor("X", [bsh, N, N, D], f32, kind="ExternalInput").ap()
    A_d = nc.dram_tensor("A", [bsh, N, N], a_dt, kind="ExternalInput").ap()
    MT_d = nc.dram_tensor("MT", [bsh, N, N], f32, kind="ExternalInput").ap()
    W1_d = nc.dram_tensor("W1Q", [128, 128], w1_dt, kind="ExternalInput").ap()
    W2_d = nc.dram_tensor("W2Q", [128, 128], w2dt, kind="ExternalInput").ap()
    B1_d = nc.dram_tensor("B1D", [128, 1], f32, kind="ExternalInput").ap()
    ID_d = nc.dram_tensor("IDN", [128, 128], f32, kind="ExternalInput").ap()
    if b2_nonzero:
        B2_d = nc.dram_tensor("B2BC", [128, G * D], f32, kind="ExternalInput").ap()
    O_d = nc.dram_tensor("OUT", [bsh, N, N, D], f32, kind="ExternalOutput").ap()

    GF = G * D  # free size of one group: 512

    with tile.TileContext(nc) as tc:
        with (
            tc.tile_pool(name="const", bufs=1) as cpool,
            tc.tile_pool(name="xb", bufs=2) as xbpool,
            tc.tile_pool(name="bmeta", bufs=2) as bmpool,
            tc.tile_pool(name="xm", bufs=2) as xmpool,
            tc.tile_pool(name="xT", bufs=2) as xTpool,
            tc.tile_pool(name="h1", bufs=2) as h1pool,
            tc.tile_pool(name="ht", bufs=2) as htpool,
            tc.tile_pool(name="ot", bufs=3) as otpool,
            tc.tile_pool(name="psT", bufs=2, space="PSUM") as psTpool,
            tc.tile_pool(name="psH1", bufs=2, space="PSUM") as psH1pool,
            tc.tile_pool(name="psH", bufs=2, space="PSUM") as psHpool,
            tc.tile_pool(name="psO", bufs=2, space="PSUM") as psOpool,
        ):
            w1q = cpool.tile([128, 128], w1_dt)
            nc.sync.dma_start(w1q[:, :], W1_d)
            w2q = cpool.tile([128, 128], w2dt)
            nc.sync.dma_start(w2q[:, :], W2_d)
            b1d = cpool.tile([128, 1], f32)
            nc.sync.dma_start(b1d[:, :], B1_d)
            idn = cpool.tile([128, 128], f32)
            nc.sync.dma_start(idn[:, :], ID_d)
            if b2_nonzero:
                b2bc = cpool.tile([128, GF], f32)
                nc.sync.dma_start(b2bc[:, :], B2_d)

            for b in range(bsh):
                # whole-batch X load: (k partitions, (i,d) free), 4 MiB
                xb = xbpool.tile([128, N * D], f32)
                nc.sync.dma_start(
                    xb[:, :].rearrange("k (i d) -> k i d", i=N),
                    X_d[b].rearrange("i k d -> k i d"),
                )
                at = bmpool.tile([128, N], a_dt, tag="at")
                nc.sync.dma_start(at[:, :], A_d[b])
                mt = bmpool.tile([128, N], f32, tag="mt")
                nc.sync.dma_start(mt[:, :], MT_d[b])

                for g in range(ng):
                    i0 = g * G
                    mtg = mt[:, i0 : i0 + G].unsqueeze(2).broadcast_to([128, G, D])

                    # mask X rows (k) for all 8 tiles in one op
                    xm = xmpool.tile([128, GF], f32)
                    nc.vector.tensor_mul(
                        xm[:, :].rearrange("k (i d) -> k i d", i=G),
                        xb[:, :].rearrange("k (i d) -> k i d", i=N)[:, i0 : i0 + G, :],
                        mtg,
                    )

                    # transpose pairs: [XmA|XmB] (k, 2*D) -> (d-stacked, k)
                    psT = psTpool.tile([128, GF], f32)
                    for p in range(G // 2):
                        nc.tensor.transpose(
                            psT[:, p * 128 : (p + 1) * 128],
                            xm[:, p * 128 : (p + 1) * 128],
                            idn[:, :],
                        )
                    xT = xTpool.tile([128, GF], w1_dt)
                    nc.scalar.copy(xT[:, :], psT[:, :])

                    # MLP layer 1: block-diag W1 computes both pair halves
                    # in one plain matmul (f32r cannot col-tile on trn2)
                    psH1 = psH1pool.tile([128, GF], f32)
                    nc.tensor.matmul(
                        psH1[:, :], w1q[:, :], xT[:, :], start=True, stop=True
                    )
                    h1t = h1pool.tile([128, GF], w2dt)
                    nc.scalar.activation(h1t[:, :], psH1[:, :], Relu, bias=b1d[:, 0:1])

                    # MLP layer 2: stationary = full h1T pair (128 rows),
                    # moving = block-diag W2 column slice ([W2;0] or [0;W2])
                    # -> h natural (k, d); plain matmuls, no tile_position
                    psH = psHpool.tile([128, GF], f32)
                    for t in range(G):
                        p, half = t // 2, t % 2
                        nc.tensor.matmul(
                            psH[:, t * D : (t + 1) * D],
                            h1t[:, p * 128 : (p + 1) * 128],
                            w2q[:, half * D : (half + 1) * D],
                            start=True,
                            stop=True,
                        )
                    if b2_nonzero:
                        nc.vector.tensor_add(psH[:, :], psH[:, :], b2bc[:, :])
                    ht = htpool.tile([128, GF], a_dt)
                    nc.scalar.activation(ht[:, :], psH[:, :], Relu)

                    # message passing: out[j, (i,d)] = sum_k A[b][k,j] * h[k, (i,d)]
                    psO = psOpool.tile([128, GF], f32)
                    nc.tensor.matmul(
                        psO[:, :], at[:, :], ht[:, :], start=True, stop=True
                    )

                    # mask output rows (j), fused into PSUM->SBUF copy
                    ot = otpool.tile([128, GF], f32)
                    nc.vector.tensor_mul(
                        ot[:, :].rearrange("j (i d) -> j i d", i=G),
                        psO[:, :].rearrange("j (i d) -> j i d", i=G),
                        mtg,
                    )
                    nc.sync.dma_start(
                        O_d[b, i0 : i0 + G].rearrange("i j d -> j i d"),
                        ot[:, :].rearrange("j (i d) -> j i d", i=G),
                    )

    nc.compile()
    nc.m = get_hw_module(nc.m)
    return nc


def kernel(X, A, mask, W1, b1, W2, b2):
    import ml_dtypes
    from concourse.bass_utils import run_bass_kernel_spmd

    cfg = dict(mm1="f32r", mm2="bf16", mm3="f32r")

    X = np.ascontiguousarray(X, dtype=np.float32)
    A = np.ascontiguousarray(A, dtype=np.float32)
    MT = np.ascontiguousarray(np.swapaxes(mask, 1, 2)).astype(np.float32)
    W1 = np.asarray(W1, dtype=np.float32)
    W2 = np.asarray(W2, dtype=np.float32)
    b1 = np.asarray(b1, dtype=np.float32)
    b2 = np.asarray(b2, dtype=np.float32)

    b2_nonzero = bool(np.any(b2 != 0.0))
    key = (b2_nonzero, tuple(sorted(cfg.items())))
    if key not in _built:
        _built[key] = _build(b2_nonzero, cfg)
    nc = _built[key]

    w2dt = ml_dtypes.bfloat16 if cfg.get("mm2", "bf16") == "bf16" else np.float32
    w1q = np.zeros((128, 128), dtype=np.float32)  # block-diag [[W1,0],[0,W1]]
    w1q[0:64, 0:64] = W1
    w1q[64:128, 64:128] = W1
    w2q = np.zeros((128, 128), dtype=np.float32)  # block-diag [[W2,0],[0,W2]]
    w2q[0:64, 0:64] = W2
    w2q[64:128, 64:128] = W2
    w2q = w2q.astype(w2dt)
    b1d = np.concatenate([b1, b1], axis=0).reshape(128, 1)
    idn = np.eye(128, dtype=np.float32)

    shared = {"W1Q": w1q, "W2Q": w2q, "B1D": b1d, "IDN": idn}
    if b2_nonzero:
        shared["B2BC"] = np.tile(b2, (128, G)).astype(np.float32)

    in_maps = []
    for c in range(NC):
        sl = slice(c * BSH, (c + 1) * BSH)
        in_maps.append({"X": X[sl], "A": A[sl], "MT": MT[sl], **shared})
    global _last_in_maps
    _last_in_maps = in_maps

    try:
        res = run_bass_kernel_spmd(nc, in_maps, core_ids=list(range(NC)))
    except Exception:
        res = run_bass_kernel_spmd(nc, in_maps, core_ids=list(range(NC)))
    out = np.concatenate([res.results[c]["OUT"] for c in range(NC)], axis=0)
    return np.ascontiguousarray(out, dtype=np.float32)



# revision 3
# speedup vs baseline: 2.3454x; 2.3454x over previous
"""Trainium2 Bass kernel for nn_NestedConv (gnn_message_passing).

Math (per b, i):
    Xm       = X[b,i] * mask[b,i,:,None]                 # (N,D), rows k masked
    h1       = relu(Xm @ W1 + b1)                        # (N,D)
    h        = relu(h1 @ W2 + b2)                        # (N,D)
    out[b,i] = (A[b].T @ h) * mask[b,i,:,None]           # (N,D), rows j masked

Key restructuring vs the obvious dataflow:
  - X is uploaded host-pre-transposed+bf16 as XT[b, (half,d), (g,p,k)] so the
    MLP contraction dim d sits on partitions with no on-chip transposes.
  - The input row-mask is deferred: rowwise MLP maps 0-rows to 0-rows when
    b1=b2=0, so masking h (natural layout) == masking X. For nonzero biases
    the exact correction  out += (A^T (1-m_i)) c^T  with c = MLP(0-row) is
    added per batch (one extra matmul) + per group (two DVE ops).
  - All matmuls bf16 (A and mask are 0/1-exact in bf16):
      mm1: stationary blockdiag[W1,W1], moving XT group     -> h1^T pairs
      mm2: stationary h1^T pair, moving blockdiag[W2,W2]    -> h natural
      mm3: stationary A[b], moving h (8 i's batched)        -> out rows j
  - Elementwise work is split across ACT/DVE/GPSIMD:
      relu1 on ACT; relu2+hmask alternates (even groups: ACT relu + GPSIMD
      mask-mul; odd groups: one fused DVE (max,mult) op); outmask on DVE.
  - Output stored bf16 in a per-batch SBUF buffer, one 2 MiB DMA per batch;
    host re-transposes to (b,i,j,d) f32.

Sharding: data-parallel over batch dim B=64 across 8 NeuronCores (8 b's each).
"""

import sys

sys.path.insert(0, "/opt/trn_rl_repo")

import numpy as np

B, N, D = 64, 128, 64
NC = 8
BSH = B // NC  # batches per core
G = 8  # root nodes i per group
NG = N // G  # groups per batch
GF = G * D  # free size of one group: 512
NP = G // 2  # stationary pairs per group: 4

_built = {}
_last_in_maps = None


def _build(bias_mode: bool, cfg: dict, bsh: int = BSH, ng: int = NG):
    import concourse.bacc as bacc
    import concourse.mybir as mybir
    from concourse import tile
    from concourse.bass_interp import get_hw_module

    f32 = mybir.dt.float32
    bf16 = mybir.dt.bfloat16
    Relu = mybir.ActivationFunctionType.Relu
    Alu = mybir.AluOpType

    relu2_mode = cfg.get("relu2", "alt")  # alt | dve | act_gps

    nc = bacc.Bacc("TRN2", target_bir_lowering=False, debug=False, num_devices=1)

    XT_d = nc.dram_tensor("XT", [bsh, 128, ng * GF], bf16, kind="ExternalInput").ap()
    A_d = nc.dram_tensor("A", [bsh, 128, 128], bf16, kind="ExternalInput").ap()
    MT_d = nc.dram_tensor("MT", [bsh, 128, 128], f32, kind="ExternalInput").ap()
    MTB_d = nc.dram_tensor("MTB", [bsh, 128, 128], bf16, kind="ExternalInput").ap()
    W1_d = nc.dram_tensor("W1Q", [128, 128], bf16, kind="ExternalInput").ap()
    W2_d = nc.dram_tensor("W2Q", [128, 128], bf16, kind="ExternalInput").ap()
    B1_d = nc.dram_tensor("B1D", [128, 1], f32, kind="ExternalInput").ap()
    if bias_mode:
        B2_d = nc.dram_tensor("B2BC", [128, GF], f32, kind="ExternalInput").ap()
        CB_d = nc.dram_tensor("CB", [128, GF], f32, kind="ExternalInput").ap()
    O_d = nc.dram_tensor("OUT", [bsh, 128, ng * GF], bf16, kind="ExternalOutput").ap()

    with tile.TileContext(nc) as tc:
        with (
            tc.tile_pool(name="const", bufs=1) as cpool,
            tc.tile_pool(name="xb", bufs=2) as xbpool,
            tc.tile_pool(name="bmeta", bufs=2) as bmpool,
            tc.tile_pool(name="ob", bufs=2) as obpool,
            tc.tile_pool(name="h1", bufs=3) as h1pool,
            tc.tile_pool(name="ht", bufs=3) as htpool,
            tc.tile_pool(name="ht0", bufs=2) as ht0pool,
            tc.tile_pool(name="psH1", bufs=2, space="PSUM") as psH1pool,
            tc.tile_pool(name="psH", bufs=2, space="PSUM") as psHpool,
            tc.tile_pool(name="psO", bufs=2, space="PSUM") as psOpool,
        ):
            ub_pool = tmpc_pool = psU_pool = None
            if bias_mode:
                ub_pool = tc.tile_pool(name="ub", bufs=2).__enter__()
                tmpc_pool = tc.tile_pool(name="tmpc", bufs=2).__enter__()
                psU_pool = tc.tile_pool(name="psU", bufs=1, space="PSUM").__enter__()

            w1q = cpool.tile([128, 128], bf16, tag="w1q")
            nc.sync.dma_start(w1q[:, :], W1_d)
            w2q = cpool.tile([128, 128], bf16, tag="w2q")
            nc.sync.dma_start(w2q[:, :], W2_d)
            b1d = cpool.tile([128, 1], f32, tag="b1d")
            nc.sync.dma_start(b1d[:, :], B1_d)
            if bias_mode:
                b2bc = cpool.tile([128, GF], f32, tag="b2bc")
                nc.sync.dma_start(b2bc[:, :], B2_d)
                cb = cpool.tile([128, GF], f32, tag="cb")
                nc.sync.dma_start(cb[:, :], CB_d)

            batch_tiles = {}

            def load_batch(b):
                if b >= bsh:
                    return
                xbT = xbpool.tile([128, ng * GF], bf16)
                nc.sync.dma_start(xbT[:, :], XT_d[b])
                at = bmpool.tile([128, 128], bf16, tag="at")
                nc.sync.dma_start(at[:, :], A_d[b])
                mt = bmpool.tile([128, 128], f32, tag="mt")
                nc.sync.dma_start(mt[:, :], MT_d[b])
                mtb = bmpool.tile([128, 128], bf16, tag="mtb")
                nc.sync.dma_start(mtb[:, :], MTB_d[b])
                batch_tiles[b] = dict(xbT=xbT, at=at, mt=mt, mtb=mtb)

            load_batch(0)

            total = bsh * ng
            ctxs = [None] * total

            def S0(i):
                b, g = divmod(i, ng)
                if g == ng // 2:
                    load_batch(b + 1)
                t = batch_tiles[b]
                ctx = dict(b=b, g=g, **t)
                if g == 0:
                    if bias_mode:
                        # U[j,i] = sum_k A[k,j] (1 - m[k,i]); c-column correction
                        omtb = ht0pool.tile([128, 128], bf16, tag="omtb")
                        nc.vector.tensor_scalar(
                            omtb[:, :], t["mt"][:, :], 1.0, -1.0,
                            Alu.subtract, Alu.mult,
                        )
                        psU = psU_pool.tile([128, 128], f32)
                        nc.tensor.matmul(
                            psU[:, :], t["at"][:, :], omtb[:, :],
                            start=True, stop=True,
                        )
                        ub = ub_pool.tile([128, 128], f32)
                        nc.scalar.copy(ub[:, :], psU[:, :])
                        batch_tiles[b]["ub"] = ub
                    obuf = obpool.tile([128, ng * GF], bf16)
                    batch_tiles[b]["obuf"] = obuf
                ctx["obuf"] = batch_tiles[b]["obuf"]
                if bias_mode:
                    ctx["ub"] = batch_tiles[b]["ub"]

                # mm1: h1^T pairs for the whole group in one matmul
                psH1 = psH1pool.tile([128, GF], f32)
                nc.tensor.matmul(
                    psH1[:, :], w1q[:, :], ctx["xbT"][:, g * GF : (g + 1) * GF],
                    start=True, stop=True,
                )
                h1t = h1pool.tile([128, GF], bf16)
                nc.scalar.activation(h1t[:, :], psH1[:, :], Relu, bias=b1d[:, 0:1])
                ctx["h1t"] = h1t
                ctxs[i] = ctx

            def S1(i):
                ctx = ctxs[i]
                g = ctx["g"]
                i0 = g * G
                h1t = ctx["h1t"]
                psH = psHpool.tile([128, GF], f32)
                for p in range(NP):
                    nc.tensor.matmul(
                        psH[:, p * 128 : (p + 1) * 128],
                        h1t[:, p * 128 : (p + 1) * 128],
                        w2q[:, :],
                        start=True, stop=True,
                    )
                if bias_mode:
                    nc.vector.tensor_add(psH[:, :], psH[:, :], b2bc[:, :])
                mtg = ctx["mt"][:, i0 : i0 + G].unsqueeze(2).broadcast_to([128, G, D])
                ht = htpool.tile([128, GF], bf16)
                ht3 = ht[:, :].rearrange("k (i d) -> k i d", i=G)
                psH3 = psH[:, :].rearrange("k (i d) -> k i d", i=G)
                use_dve = relu2_mode == "dve" or (relu2_mode == "alt" and g % 2 == 1)
                if use_dve:
                    # ht = relu(psH) * m  ==  (psH max 0) * m, one fused DVE op
                    nc.vector.scalar_tensor_tensor(
                        ht3, psH3, 0.0, mtg, Alu.max, Alu.mult
                    )
                else:
                    ht0 = ht0pool.tile([128, GF], bf16, tag="ht0")
                    nc.scalar.activation(ht0[:, :], psH[:, :], Relu)
                    mtgb = (
                        ctx["mtb"][:, i0 : i0 + G]
                        .unsqueeze(2)
                        .broadcast_to([128, G, D])
                    )
                    nc.gpsimd.tensor_mul(
                        ht3, ht0[:, :].rearrange("k (i d) -> k i d", i=G), mtgb
                    )
                ctx["ht"] = ht

            def S2(i):
                ctx = ctxs[i]
                b, g = ctx["b"], ctx["g"]
                i0 = g * G
                psO = psOpool.tile([128, GF], f32)
                nc.tensor.matmul(
                    psO[:, :], ctx["at"][:, :], ctx["ht"][:, :],
                    start=True, stop=True,
                )
                psO3 = psO[:, :].rearrange("j (i d) -> j i d", i=G)
                mtg = ctx["mt"][:, i0 : i0 + G].unsqueeze(2).broadcast_to([128, G, D])
                if bias_mode:
                    tmpc = tmpc_pool.tile([128, GF], f32)
                    ubg = (
                        ctx["ub"][:, i0 : i0 + G]
                        .unsqueeze(2)
                        .broadcast_to([128, G, D])
                    )
                    nc.vector.tensor_mul(
                        tmpc[:, :].rearrange("j (i d) -> j i d", i=G),
                        ubg,
                        cb[:, :].rearrange("j (i d) -> j i d", i=G),
                    )
                    nc.vector.tensor_add(psO[:, :], psO[:, :], tmpc[:, :])
                ot3 = (
                    ctx["obuf"][:, g * GF : (g + 1) * GF]
                    .rearrange("j (i d) -> j i d", i=G)
                )
                nc.vector.tensor_mul(ot3, psO3, mtg)
                if g == ng - 1:
                    nc.sync.dma_start(O_d[b], ctx["obuf"][:, :])

            for i in range(total):
                S0(i)
                if i >= 1:
                    S1(i - 1)
                if i >= 2:
                    S2(i - 2)
            S1(total - 1)
            S2(total - 2)
            S2(total - 1)

            if bias_mode:
                ub_pool.__exit__(None, None, None)
                tmpc_pool.__exit__(None, None, None)
                psU_pool.__exit__(None, None, None)

    nc.compile()
    nc.m = get_hw_module(nc.m)
    return nc


def kernel(X, A, mask, W1, b1, W2, b2):
    import ml_dtypes
    from concourse.bass_utils import run_bass_kernel_spmd

    bf = ml_dtypes.bfloat16
    cfg = dict(relu2="alt")

    X = np.asarray(X, dtype=np.float32)
    A = np.asarray(A, dtype=np.float32)
    mask = np.asarray(mask)
    W1 = np.asarray(W1, dtype=np.float32)
    W2 = np.asarray(W2, dtype=np.float32)
    b1 = np.asarray(b1, dtype=np.float32)
    b2 = np.asarray(b2, dtype=np.float32)

    bias_mode = bool(np.any(b1 != 0.0) or np.any(b2 != 0.0))
    key = (bias_mode, tuple(sorted(cfg.items())))
    if key not in _built:
        _built[key] = _build(bias_mode, cfg)
    nc = _built[key]

    # XT[b, (half,d), (g,p,k)] for i = 8g + 2p + half
    XT = np.ascontiguousarray(
        X.reshape(B, NG, NP, 2, N, D).transpose(0, 3, 5, 1, 2, 4)
    ).reshape(B, 128, NG * GF).astype(bf)
    Ab = A.astype(bf)
    MTf = np.ascontiguousarray(np.swapaxes(mask, 1, 2)).astype(np.float32)
    MTb = MTf.astype(bf)

    w1q = np.zeros((128, 128), dtype=np.float32)
    w1q[0:64, 0:64] = W1
    w1q[64:128, 64:128] = W1
    w2q = np.zeros((128, 128), dtype=np.float32)
    w2q[0:64, 0:64] = W2
    w2q[64:128, 64:128] = W2
    b1d = np.concatenate([b1, b1], axis=0).reshape(128, 1).astype(np.float32)

    shared = {
        "W1Q": w1q.astype(bf),
        "W2Q": w2q.astype(bf),
        "B1D": b1d,
    }
    if bias_mode:
        c = np.maximum(np.maximum(b1, 0.0) @ W2 + b2, 0.0).astype(np.float32)
        shared["B2BC"] = np.tile(b2, (128, G)).astype(np.float32)
        shared["CB"] = np.tile(c, (128, G)).astype(np.float32)

    in_maps = []
    for cid in range(NC):
        sl = slice(cid * BSH, (cid + 1) * BSH)
        in_maps.append(
            {"XT": XT[sl], "A": Ab[sl], "MT": MTf[sl], "MTB": MTb[sl], **shared}
        )
    global _last_in_maps
    _last_in_maps = in_maps

    try:
        res = run_bass_kernel_spmd(nc, in_maps, core_ids=list(range(NC)))
    except Exception:
        res = run_bass_kernel_spmd(nc, in_maps, core_ids=list(range(NC)))
    OT = np.concatenate([res.results[c]["OUT"] for c in range(NC)], axis=0)
    # OT[b, j, (g, ig, d)] -> out[b, i=8g+ig, j, d]
    out = (
        OT.astype(np.float32)
        .reshape(B, N, NG, G, D)
        .transpose(0, 2, 3, 1, 4)
        .reshape(B, N, N, D)
    )
    return np.ascontiguousarray(out)


# revision 11
# speedup vs baseline: 2.3776x; 1.0137x over previous
"""Trainium2 Bass kernel for nn_NestedConv (gnn_message_passing).

Math (per b, i):
    Xm       = X[b,i] * mask[b,i,:,None]                 # (N,D), rows k masked
    h1       = relu(Xm @ W1 + b1)                        # (N,D)
    h        = relu(h1 @ W2 + b2)                        # (N,D)
    out[b,i] = (A[b].T @ h) * mask[b,i,:,None]           # (N,D), rows j masked

Key restructuring vs the obvious dataflow:
  - X is uploaded host-pre-transposed+bf16 as XT[b, (half,d), (g,p,k)] so the
    MLP contraction dim d sits on partitions with no on-chip transposes.
  - The input row-mask is deferred: rowwise MLP maps 0-rows to 0-rows when
    b1=b2=0, so masking h (natural layout) == masking X. For nonzero biases
    the exact correction  out += (A^T (1-m_i)) c^T  with c = MLP(0-row) is
    added per batch (one extra matmul) + per group (two DVE ops).
  - All matmuls bf16 (A and mask are 0/1-exact in bf16):
      mm1: stationary blockdiag[W1,W1], moving XT group     -> h1^T pairs
      mm2: stationary h1^T pair, moving blockdiag[W2,W2]    -> h natural
      mm3: stationary A[b], moving h (8 i's batched)        -> out rows j
  - Elementwise work is split across ACT/DVE/GPSIMD:
      relu1 on ACT; relu2+hmask alternates (even groups: ACT relu + GPSIMD
      mask-mul; odd groups: one fused DVE (max,mult) op); outmask on DVE.
  - Output stored bf16 in a per-batch SBUF buffer, one 2 MiB DMA per batch;
    host re-transposes to (b,i,j,d) f32.

Sharding: data-parallel over batch dim B=64 across 8 NeuronCores (8 b's each).
"""

import sys

sys.path.insert(0, "/opt/trn_rl_repo")

import numpy as np

B, N, D = 64, 128, 64
NC = 8
BSH = B // NC  # batches per core
G = 8  # root nodes i per group
NG = N // G  # groups per batch
GF = G * D  # free size of one group: 512
NP = G // 2  # stationary pairs per group: 4

_built = {}
_last_in_maps = None


def _build(bias_mode: bool, cfg: dict, bsh: int = BSH, ng: int = NG):
    import concourse.bacc as bacc
    import concourse.mybir as mybir
    from concourse import tile
    from concourse.bass_interp import get_hw_module

    f32 = mybir.dt.float32
    bf16 = mybir.dt.bfloat16
    Relu = mybir.ActivationFunctionType.Relu
    Alu = mybir.AluOpType

    relu2_mode = cfg.get("relu2", "alt")  # alt | dve | act_gps

    nc = bacc.Bacc("TRN2", target_bir_lowering=False, debug=False, num_devices=1)

    XT_d = nc.dram_tensor("XT", [bsh, 128, ng * GF], bf16, kind="ExternalInput").ap()
    A_d = nc.dram_tensor("A", [bsh, 128, 128], bf16, kind="ExternalInput").ap()
    MT_d = nc.dram_tensor("MT", [bsh, 128, 128], f32, kind="ExternalInput").ap()
    MTB_d = nc.dram_tensor("MTB", [bsh, 128, 128], bf16, kind="ExternalInput").ap()
    W1_d = nc.dram_tensor("W1Q", [128, 128], bf16, kind="ExternalInput").ap()
    W2_d = nc.dram_tensor("W2Q", [128, 128], bf16, kind="ExternalInput").ap()
    B1_d = nc.dram_tensor("B1D", [128, 1], f32, kind="ExternalInput").ap()
    if bias_mode:
        B2_d = nc.dram_tensor("B2BC", [128, GF], f32, kind="ExternalInput").ap()
        CB_d = nc.dram_tensor("CB", [128, GF], f32, kind="ExternalInput").ap()
    O_d = nc.dram_tensor("OUT", [bsh, 128, ng * GF], bf16, kind="ExternalOutput").ap()

    ocst = cfg.get("ochunk", 4)  # groups per output-store chunk

    with tile.TileContext(nc) as tc:
        with (
            tc.tile_pool(name="const", bufs=1) as cpool,
            tc.tile_pool(name="xb", bufs=2) as xbpool,
            tc.tile_pool(name="bmeta", bufs=2) as bmpool,
            tc.tile_pool(name="ob", bufs=2) as obpool,
            tc.tile_pool(name="h1", bufs=2) as h1pool,
            tc.tile_pool(name="ht", bufs=3) as htpool,
            tc.tile_pool(name="ht0", bufs=2) as ht0pool,
            tc.tile_pool(name="psH1", bufs=2, space="PSUM") as psH1pool,
            tc.tile_pool(name="psH", bufs=2, space="PSUM") as psHpool,
            tc.tile_pool(name="psO", bufs=2, space="PSUM") as psOpool,
        ):
            ub_pool = tmpc_pool = psU_pool = None
            if bias_mode:
                ub_pool = tc.tile_pool(name="ub", bufs=2).__enter__()
                tmpc_pool = tc.tile_pool(name="tmpc", bufs=2).__enter__()
                psU_pool = tc.tile_pool(name="psU", bufs=1, space="PSUM").__enter__()

            w1q = cpool.tile([128, 128], bf16, tag="w1q")
            nc.sync.dma_start(w1q[:, :], W1_d)
            w2q = cpool.tile([128, 128], bf16, tag="w2q")
            nc.sync.dma_start(w2q[:, :], W2_d)
            b1d = cpool.tile([128, 1], f32, tag="b1d")
            nc.sync.dma_start(b1d[:, :], B1_d)
            if bias_mode:
                b2bc = cpool.tile([128, GF], f32, tag="b2bc")
                nc.sync.dma_start(b2bc[:, :], B2_d)
                cb = cpool.tile([128, GF], f32, tag="cb")
                nc.sync.dma_start(cb[:, :], CB_d)

            batch_tiles = {}

            def load_batch(b):
                if b >= bsh:
                    return
                xbT = xbpool.tile([128, ng * GF], bf16)
                nc.sync.dma_start(xbT[:, :], XT_d[b])
                at = bmpool.tile([128, 128], bf16, tag="at")
                nc.sync.dma_start(at[:, :], A_d[b])
                mt = bmpool.tile([128, 128], f32, tag="mt")
                nc.sync.dma_start(mt[:, :], MT_d[b])
                mtb = bmpool.tile([128, 128], bf16, tag="mtb")
                nc.sync.dma_start(mtb[:, :], MTB_d[b])
                batch_tiles[b] = dict(xbT=xbT, at=at, mt=mt, mtb=mtb)

            load_batch(0)

            total = bsh * ng
            ctxs = [None] * total

            # super-group span: 2 groups share one psH1 tile + one relu1.
            # bias mode needs a PSUM bank for psU, so it stays at span 1.
            sgn = 1 if bias_mode else 2

            def S0(i):
                # one S0 covers a super-group of sgn consecutive groups:
                # sgn N=512 matmuls into one PSUM tile, a single wide
                # relu1, shared h1t tile.
                b, g = divmod(i, ng)
                if g == ng // 4:
                    load_batch(b + 1)
                t = batch_tiles[b]
                if g == 0:
                    if bias_mode:
                        # U[j,i] = sum_k A[k,j] (1 - m[k,i]); c-column correction
                        omtb = ht0pool.tile([128, 128], bf16, tag="omtb")
                        nc.vector.tensor_scalar(
                            omtb[:, :], t["mt"][:, :], 1.0, -1.0,
                            Alu.subtract, Alu.mult,
                        )
                        psU = psU_pool.tile([128, 128], f32)
                        nc.tensor.matmul(
                            psU[:, :], t["at"][:, :], omtb[:, :],
                            start=True, stop=True,
                        )
                        ub = ub_pool.tile([128, 128], f32)
                        nc.scalar.copy(ub[:, :], psU[:, :])
                        batch_tiles[b]["ub"] = ub
                    obuf = obpool.tile([128, ng * GF], bf16)
                    batch_tiles[b]["obuf"] = obuf

                psH1 = psH1pool.tile([128, sgn * GF], f32)
                for s in range(sgn):
                    gs = g + s
                    nc.tensor.matmul(
                        psH1[:, s * GF : (s + 1) * GF],
                        w1q[:, :],
                        t["xbT"][:, gs * GF : (gs + 1) * GF],
                        start=True, stop=True,
                    )
                h1t = h1pool.tile([128, sgn * GF], bf16)
                nc.scalar.activation(h1t[:, :], psH1[:, :], Relu, bias=b1d[:, 0:1])
                for s in range(sgn):
                    ctx = dict(b=b, g=g + s, **t)
                    ctx["obuf"] = batch_tiles[b]["obuf"]
                    if bias_mode:
                        ctx["ub"] = batch_tiles[b]["ub"]
                    ctx["h1t"] = h1t
                    ctx["h1off"] = s * GF
                    ctxs[i + s] = ctx

            def S1(i):
                ctx = ctxs[i]
                g = ctx["g"]
                i0 = g * G
                h1t = ctx["h1t"]
                off = ctx["h1off"]
                psH = psHpool.tile([128, GF], f32)
                for p in range(NP):
                    nc.tensor.matmul(
                        psH[:, p * 128 : (p + 1) * 128],
                        h1t[:, off + p * 128 : off + (p + 1) * 128],
                        w2q[:, :],
                        start=True, stop=True,
                    )
                if bias_mode:
                    nc.vector.tensor_add(psH[:, :], psH[:, :], b2bc[:, :])
                mtg = ctx["mt"][:, i0 : i0 + G].unsqueeze(2).broadcast_to([128, G, D])
                ht = htpool.tile([128, GF], bf16)
                ht3 = ht[:, :].rearrange("k (i d) -> k i d", i=G)
                psH3 = psH[:, :].rearrange("k (i d) -> k i d", i=G)
                use_dve = relu2_mode == "dve" or (relu2_mode == "alt" and g % 2 == 1)
                if use_dve:
                    # ht = relu(psH) * m  ==  (psH max 0) * m, one fused DVE op
                    nc.vector.scalar_tensor_tensor(
                        ht3, psH3, 0.0, mtg, Alu.max, Alu.mult
                    )
                else:
                    ht0 = ht0pool.tile([128, GF], bf16, tag="ht0")
                    nc.scalar.activation(ht0[:, :], psH[:, :], Relu)
                    mtgb = (
                        ctx["mtb"][:, i0 : i0 + G]
                        .unsqueeze(2)
                        .broadcast_to([128, G, D])
                    )
                    nc.gpsimd.tensor_mul(
                        ht3, ht0[:, :].rearrange("k (i d) -> k i d", i=G), mtgb
                    )
                ctx["ht"] = ht

            def S2(i):
                ctx = ctxs[i]
                b, g = ctx["b"], ctx["g"]
                i0 = g * G
                psO = psOpool.tile([128, GF], f32)
                nc.tensor.matmul(
                    psO[:, :], ctx["at"][:, :], ctx["ht"][:, :],
                    start=True, stop=True,
                )
                psO3 = psO[:, :].rearrange("j (i d) -> j i d", i=G)
                mtg = ctx["mt"][:, i0 : i0 + G].unsqueeze(2).broadcast_to([128, G, D])
                if bias_mode:
                    tmpc = tmpc_pool.tile([128, GF], f32)
                    ubg = (
                        ctx["ub"][:, i0 : i0 + G]
                        .unsqueeze(2)
                        .broadcast_to([128, G, D])
                    )
                    nc.vector.tensor_mul(
                        tmpc[:, :].rearrange("j (i d) -> j i d", i=G),
                        ubg,
                        cb[:, :].rearrange("j (i d) -> j i d", i=G),
                    )
                    nc.vector.tensor_add(psO[:, :], psO[:, :], tmpc[:, :])
                ot3 = (
                    ctx["obuf"][:, g * GF : (g + 1) * GF]
                    .rearrange("j (i d) -> j i d", i=G)
                )
                nc.vector.tensor_mul(ot3, psO3, mtg)
                if (g + 1) % ocst == 0:
                    c0 = (g + 1 - ocst) * GF
                    c1 = (g + 1) * GF
                    nc.sync.dma_start(O_d[b][:, c0:c1], ctx["obuf"][:, c0:c1])

            for i in range(total):
                if i % sgn == 0:
                    S0(i)
                if i >= 1:
                    S1(i - 1)
                if i >= 2:
                    S2(i - 2)
            S1(total - 1)
            S2(total - 2)
            S2(total - 1)

            if bias_mode:
                ub_pool.__exit__(None, None, None)
                tmpc_pool.__exit__(None, None, None)
                psU_pool.__exit__(None, None, None)

    nc.compile()
    nc.m = get_hw_module(nc.m)
    return nc


def kernel(X, A, mask, W1, b1, W2, b2):
    import ml_dtypes
    from concourse.bass_utils import run_bass_kernel_spmd

    bf = ml_dtypes.bfloat16
    cfg = dict(relu2="alt", ochunk=4)

    X = np.asarray(X, dtype=np.float32)
    A = np.asarray(A, dtype=np.float32)
    mask = np.asarray(mask)
    W1 = np.asarray(W1, dtype=np.float32)
    W2 = np.asarray(W2, dtype=np.float32)
    b1 = np.asarray(b1, dtype=np.float32)
    b2 = np.asarray(b2, dtype=np.float32)

    bias_mode = bool(np.any(b1 != 0.0) or np.any(b2 != 0.0))
    key = (bias_mode, tuple(sorted(cfg.items())))
    if key not in _built:
        _built[key] = _build(bias_mode, cfg)
    nc = _built[key]

    # XT[b, (half,d), (g,p,k)] for i = 8g + 2p + half
    XT = np.ascontiguousarray(
        X.reshape(B, NG, NP, 2, N, D).transpose(0, 3, 5, 1, 2, 4)
    ).reshape(B, 128, NG * GF).astype(bf)
    Ab = A.astype(bf)
    MTf = np.ascontiguousarray(np.swapaxes(mask, 1, 2)).astype(np.float32)
    MTb = MTf.astype(bf)

    w1q = np.zeros((128, 128), dtype=np.float32)
    w1q[0:64, 0:64] = W1
    w1q[64:128, 64:128] = W1
    w2q = np.zeros((128, 128), dtype=np.float32)
    w2q[0:64, 0:64] = W2
    w2q[64:128, 64:128] = W2
    b1d = np.concatenate([b1, b1], axis=0).reshape(128, 1).astype(np.float32)

    shared = {
        "W1Q": w1q.astype(bf),
        "W2Q": w2q.astype(bf),
        "B1D": b1d,
    }
    if bias_mode:
        c = np.maximum(np.maximum(b1, 0.0) @ W2 + b2, 0.0).astype(np.float32)
        shared["B2BC"] = np.tile(b2, (128, G)).astype(np.float32)
        shared["CB"] = np.tile(c, (128, G)).astype(np.float32)

    in_maps = []
    for cid in range(NC):
        sl = slice(cid * BSH, (cid + 1) * BSH)
        in_maps.append(
            {"XT": XT[sl], "A": Ab[sl], "MT": MTf[sl], "MTB": MTb[sl], **shared}
        )
    global _last_in_maps
    _last_in_maps = in_maps

    try:
        res = run_bass_kernel_spmd(nc, in_maps, core_ids=list(range(NC)))
    except Exception:
        res = run_bass_kernel_spmd(nc, in_maps, core_ids=list(range(NC)))
    OT = np.concatenate([res.results[c]["OUT"] for c in range(NC)], axis=0)
    # OT[b, j, (g, ig, d)] -> out[b, i=8g+ig, j, d]
    out = (
        OT.astype(np.float32)
        .reshape(B, N, NG, G, D)
        .transpose(0, 2, 3, 1, 4)
        .reshape(B, N, N, D)
    )
    return np.ascontiguousarray(out)
